# revision 39
# baseline (speedup 1.0000x reference)
"""Trainium2 Bass kernel for nn_C3D_15470472200649.

C3D video encoder (8 conv3d layers + fc6/fc7) + pairwise cosine + Sinkhorn OT.
Sharding: data-parallel over the 24 clips (3 per core) for the encoder;
fc6/fc7 sharded over output features (512/core); features exchanged with
AllGather; the tiny OT stage is replicated on every core.

conv1 runs in fp16 with host-side im2col packing two adjacent output columns
into the 128 output partitions (K=108 = 3cin x 3x3 taps x 4-wide windows).
conv2..conv5b run as fp8(e4m3) DoubleRow matmuls: each instruction contracts
TWO conv taps (constant address delta on the pair axis) at 0.5 cycles/row.
Taps are paired so the pair-axis byte delta is EVEN (odd deltas fault on
hardware). Accuracy is held by an activation residual split: activations are
stored as x8 + r8 (both fp8, r8 = fp16 value minus x8), and every conv
accumulates w8*x8 + w8*r8 into fp32 PSUM; weights are single fp8 (their
quantization error is shared between query and support branches and largely
cancels in the cosine). Inter-layer volumes are zero-padded fp8 pairs;
windows span padded rows flat-contiguously so the DoubleRow moving operand
stays 3-dim. conv1/conv2 are software-pipelined through a 6-frame SBUF ring;
the tail uses one AllGather (conv features) + one AllReduce (fc7 partials,
fc7 being linear in the sharded fc6 blocks); the tiny Sinkhorn runs
replicated on every core.
"""

import dataclasses
import math
import numpy as np
import ml_dtypes

N_CORES = 8
SEGLEN, CIN, H0, W0 = 16, 3, 112, 112
REG, COST_ALPHA = 7.0, 0.4
SINK_ITERS = 12          # converges exactly by ~10; reference runs 100
BN = np.float32(1.0 / np.sqrt(1.0 + 1e-5))
F16 = np.float16
F8 = ml_dtypes.float8_e4m3


def _pos_cost():
    t = np.arange(4, dtype=np.float32) / 4.0
    d2 = (t[:, None] - t[None, :]) ** 2
    return np.exp(-(1.0 / (d2 + 1.0))).astype(np.float32)


# ---------------- host-side preparation ----------------

def _tap_list(KB):
    taps = []
    for kb in range(KB):
        for kd in range(3):
            for kh in range(3):
                for kw in range(3):
                    taps.append((kb, kd, kh, kw))
    return taps


def _dr_pairs(KB, PD, PH, PW):
    """Pair taps so the rhs pair-axis delta is EVEN (odd deltas fault on HW).

    Returns list of (tap_a, tap_b_or_None) index pairs into _tap_list(KB).
    """
    taps = _tap_list(KB)
    V = PD * PH * PW
    off = [((kb * PD + kd) * PH + kh) * PW + kw for kb, kd, kh, kw in taps]
    ev = [i for i in range(len(taps)) if off[i] % 2 == 0]
    od = [i for i in range(len(taps)) if off[i] % 2 == 1]
    pairs = []
    for lst in (ev, od):
        for j in range(0, len(lst) - 1, 2):
            pairs.append((lst[j], lst[j + 1]))
        if len(lst) % 2:
            pairs.append((lst[-1], None))
    return pairs


def _conv_w_dr(w, KB, MB, dims):
    """w (Cout, Cin, 3,3,3) -> [128, MB*npairs*2*128] f8 DoubleRow pairs."""
    Cout, Cin = w.shape[:2]
    wm = w.transpose(2, 3, 4, 1, 0).reshape(3, 3, 3, Cin, Cout)
    taps = _tap_list(KB)
    pairs = _dr_pairs(KB, *dims)
    out = np.zeros((128, MB, len(pairs), 2, 128), np.float32)
    PK = Cin // KB
    PM = Cout // MB
    for mb in range(MB):
        for pi, (ta, tb) in enumerate(pairs):
            for i, ti in enumerate((ta, tb)):
                if ti is None:
                    continue  # zero pad slot
                kb, kd, kh, kw = taps[ti]
                out[:PK, mb, pi, i, :PM] = wm[kd, kh, kw,
                                              kb * PK:(kb + 1) * PK,
                                              mb * PM:(mb + 1) * PM]
    return out.reshape(128, -1).astype(F8)


def _conv2_slot_pairs():
    """conv2 DR slot pairing: full slots (kw0 lower/kw2 upper, even offset)
    pair together; half slots (kw1, odd offset) pair together."""
    full = [(kd, kh, 0) for kd in range(3) for kh in range(3)]
    half = [(kd, kh, 1) for kd in range(3) for kh in range(3)]
    pairs = []
    for lst in (full, half):
        for j in range(0, 8, 2):
            pairs.append((lst[j], lst[j + 1]))
        pairs.append((lst[8], None))
    return pairs


def _conv2_w_dr(w2):
    """conv2 (128, 64, 3,3,3) -> [128, 9*2*128] f8.

    9 DR pairs, one per (kd, kh): slot0 = partition-packed (kw0 lower 64,
    kw2 upper 64); slot1 = kw1 lower, zero upper. Pair delta in rhs = +1.
    """
    wm = w2.transpose(2, 3, 4, 1, 0)  # (kd, kh, kw, 64, 128)
    out = np.zeros((128, 10, 2, 128), np.float32)
    for pi, (sa, sb) in enumerate(_conv2_slot_pairs()):
        for i, s in enumerate((sa, sb)):
            if s is None:
                continue
            kd, kh, half = s
            if half:
                out[:64, pi, i, :] = wm[kd, kh, 1]
            else:
                out[:64, pi, i, :] = wm[kd, kh, 0]
                out[64:, pi, i, :] = wm[kd, kh, 2]
    return out.reshape(128, -1).astype(F8)


def _conv1_w_packed(w1):
    """(64, 3, 3,3,3) -> [108, 128] f16; col = pos*64 + ch; row =
    ((c*3+kd)*3+kh)*4 + kw4; value = w1[ch,c,kd,kh,kw4-pos] (0 outside)."""
    out = np.zeros((108, 128), np.float32)
    for c in range(CIN):
        for kd in range(3):
            for kh in range(3):
                for kw4 in range(4):
                    r = ((c * 3 + kd) * 3 + kh) * 4 + kw4
                    for pos in range(2):
                        kw = kw4 - pos
                        if 0 <= kw <= 2:
                            out[r, pos * 64:pos * 64 + 64] = w1[:, c, kd, kh, kw]
    return out.astype(F16)


def _im2col_clip_packed(clip):
    """clip (3, 16, 112, 112) -> [108, 16*112*56] f16, cols (d, h, w')."""
    xp = np.zeros((CIN, SEGLEN + 2, H0 + 2, W0 + 2), np.float32)
    xp[:, 1:-1, 1:-1, 1:-1] = clip
    out = np.empty((108, SEGLEN * H0 * 56), F16)
    for c in range(CIN):
        for kd in range(3):
            for kh in range(3):
                for kw4 in range(4):
                    r = ((c * 3 + kd) * 3 + kh) * 4 + kw4
                    sl = xp[c, kd:kd + SEGLEN, kh:kh + H0, kw4:kw4 + 111:2]
                    out[r] = sl.reshape(-1).astype(F16)
    return out


def _prep_inputs(inputs):
    sup = np.asarray(inputs["support_set"], np.float32)
    qry = np.asarray(inputs["query_set"], np.float32)
    sp = np.swapaxes(sup, 2, 3).reshape(-1, CIN, SEGLEN, H0, W0)
    qr = np.swapaxes(qry, 2, 3).reshape(-1, CIN, SEGLEN, H0, W0)
    clips = np.concatenate([sp, qr], 0)  # 0-11 support, 12-23 query

    w1p = _conv1_w_packed(np.asarray(inputs["conv1_w"], np.float32))
    w2dr = _conv2_w_dr(np.asarray(inputs["conv2_w"], np.float32))
    w3a = _conv_w_dr(np.asarray(inputs["conv3a_w"], np.float32), 1, 2,
                     (10, 30, 30))
    w3b = _conv_w_dr(np.asarray(inputs["conv3b_w"], np.float32), 2, 2,
                     (10, 30, 30))
    w4a = _conv_w_dr(np.asarray(inputs["conv4a_w"], np.float32), 2, 4,
                     (6, 16, 16))
    w4b = _conv_w_dr(np.asarray(inputs["conv4b_w"], np.float32), 4, 4,
                     (6, 16, 16))
    w5a = _conv_w_dr(np.asarray(inputs["conv5a_w"], np.float32), 4, 4,
                     (4, 9, 9))
    w5b = _conv_w_dr(np.asarray(inputs["conv5b_w"], np.float32), 4, 4,
                     (4, 9, 9))
    fc6w = np.asarray(inputs["fc6_w"], np.float32)
    fc7w = np.asarray(inputs["fc7_w"], np.float32)

    def bc(b, scale, blocks):
        cols = np.zeros((128, blocks), np.float32)
        b = np.asarray(b, np.float32) * scale
        n = b.size // blocks
        for m in range(blocks):
            cols[:n, m] = b[m * n:(m + 1) * n]
        return cols

    b1 = np.asarray(inputs["conv1_b"], np.float32) * BN
    b1col = np.concatenate([b1, b1])[:, None]  # replicated for 2-pos packing

    pos = _pos_cost()
    bmat = np.zeros((9, 16), np.float32)
    bmat[:] = (math.log(4.0) - REG - REG * COST_ALPHA * pos).reshape(-1)[None]
    eye24 = np.eye(24, dtype=np.float32)

    def fc6_lhsT(w_slice):
        # feature f=(mbk*128+p)*16+hw <-> kb=(mbk,h,w), partition p
        a = w_slice.reshape(4, 128, 4, 128, 16)      # (omb, o, mbk, p, hw)
        a = a.transpose(3, 0, 2, 4, 1)               # (p, omb, mbk, hw, o)
        return a.reshape(128, 4 * 64 * 128).astype(F16)

    def fc7_lhsT(w_cols):
        # w_cols (4096, 512): local K blocks kbl of 128
        a = w_cols.reshape(32, 128, 4, 128)          # (omb, o, kbl, p)
        a = a.transpose(3, 0, 2, 1)                  # (p, omb, kbl, o)
        return a.reshape(128, 32 * 4 * 128).astype(F16)

    in_maps = []
    for core in range(N_CORES):
        patches = np.concatenate(
            [_im2col_clip_packed(clips[core * 3 + c]) for c in range(3)],
            axis=1)
        r0, r1 = core * 512, (core + 1) * 512
        bias = np.concatenate([
            b1col, bc(inputs["conv2_b"], BN, 1),
            bc(inputs["conv3a_b"], 1.0, 2), bc(inputs["conv3b_b"], BN, 2),
            bc(inputs["conv4a_b"], 1.0, 4), bc(inputs["conv4b_b"], BN, 4),
            bc(inputs["conv5a_b"], 1.0, 4), bc(inputs["conv5b_b"], BN, 4),
            bc(np.asarray(inputs["fc6_b"])[r0:r1], BN, 4),
            bc(np.asarray(inputs["fc7_b"]), BN, 32),
        ], axis=1)
        in_maps.append({
            "patches": patches,
            "w1": w1p, "w2dr": w2dr,
            "w3a": w3a, "w3b": w3b, "w4a": w4a, "w4b": w4b,
            "w5a": w5a, "w5b": w5b,
            "fc6w": fc6_lhsT(fc6w[r0:r1]),
            "fc7w": fc7_lhsT(fc7w[:, r0:r1]),
            "bias": bias, "bmat": bmat, "eye24": eye24,
        })
    return in_maps


# ---------------- device program ----------------

_BUILD_CACHE = {}


def _ap_dims(ap_obj, dims):
    """Replace the free dims of a 1-elem AP base with explicit [stride,size]."""
    return dataclasses.replace(ap_obj, ap=[list(ap_obj.ap[0])] + dims)


def _build():
    import contextlib
    import concourse.bass as bass  # noqa: F401
    import concourse.tile as tile
    from concourse import bacc, mybir

    f16 = mybir.dt.float16
    f32 = mybir.dt.float32
    f8 = mybir.dt.float8e4
    AF = mybir.ActivationFunctionType
    ALU = mybir.AluOpType
    DR = mybir.MatmulPerfMode.DoubleRow

    nc = bacc.Bacc("TRN2", target_bir_lowering=False, debug=False,
                   num_devices=N_CORES)

    din = {}
    din["patches"] = nc.dram_tensor("patches", [108, 3 * SEGLEN * H0 * 56],
                                    f16, kind="ExternalInput")
    din["w1"] = nc.dram_tensor("w1", [108, 128], f16, kind="ExternalInput")
    din["w2dr"] = nc.dram_tensor("w2dr", [128, 10 * 2 * 128], f8,
                                 kind="ExternalInput")
    CONVS = {
        # name: (KB, MB, D, Hs, Ws, pool, bias_col, scale)
        "w3a": (1, 2, 8, 28, 28, None, 2, 1.0),
        "w3b": (2, 2, 8, 28, 28, "222", 4, float(BN)),
        "w4a": (2, 4, 4, 14, 14, None, 6, 1.0),
        "w4b": (4, 4, 4, 14, 14, "222", 10, float(BN)),
        "w5a": (4, 4, 2, 7, 7, None, 14, 1.0),
        "w5b": (4, 4, 2, 7, 7, "5", 18, float(BN)),
    }
    for nm, (kb, mb) in [(k, (v[0], v[1])) for k, v in CONVS.items()]:
        npr = (kb * 27 + 1) // 2
        din[nm] = nc.dram_tensor(nm, [128, mb * npr * 2 * 128], f8,
                                 kind="ExternalInput")
    din["fc6w"] = nc.dram_tensor("fc6w", [128, 4 * 64 * 128], f16,
                                 kind="ExternalInput")
    din["fc7w"] = nc.dram_tensor("fc7w", [128, 4 * 32 * 128], f16,
                                 kind="ExternalInput")
    din["bias"] = nc.dram_tensor("bias", [128, 58], f32, kind="ExternalInput")
    din["bmat"] = nc.dram_tensor("bmat", [9, 16], f32, kind="ExternalInput")
    din["eye24"] = nc.dram_tensor("eye24", [24, 24], f32, kind="ExternalInput")
    out_d = nc.dram_tensor("out", [9, 1], f32, kind="ExternalOutput")

    with tile.TileContext(nc) as tc:
        ctx = contextlib.ExitStack()
        with ctx:
            dram = ctx.enter_context(tc.tile_pool(name="dram", bufs=1,
                                                  space="DRAM"))
            ps = ctx.enter_context(tc.tile_pool(name="ps", bufs=8,
                                                space="PSUM"))
            const_p = ctx.enter_context(tc.tile_pool(name="const", bufs=1))
            stp = ctx.enter_context(tc.tile_pool(name="stp", bufs=4))
            pool1 = ctx.enter_context(tc.tile_pool(name="pool1", bufs=2))
            pool2 = ctx.enter_context(tc.tile_pool(name="pool2", bufs=4))
            dstp = ctx.enter_context(tc.tile_pool(name="dstp", bufs=3))
            stgp = ctx.enter_context(tc.tile_pool(name="stgp", bufs=1))
            sk = ctx.enter_context(tc.tile_pool(name="sk", bufs=1))

            bias_sb = const_p.tile([128, 58], f32)
            nc.sync.dma_start(bias_sb[:], din["bias"][:])

            # elementwise engine alternation (DVE : gpsimd = 2 : 1)
            ew_state = [0]

            def ew():
                ew_state[0] += 1
                return nc.vector

            # DRAM inter-layer volumes (padded fp8 x8/r8 pairs)
            VOLS = {
                "x3": (1, 10 * 30 * 30), "x3b": (2, 10 * 30 * 30),
                "x4": (2, 6 * 16 * 16), "x4b": (4, 6 * 16 * 16),
                "x5": (4, 4 * 9 * 9), "x5b": (4, 4 * 9 * 9),
            }
            vols = {}
            for nm, (kb, v) in VOLS.items():
                vols[nm] = [(dram.tile([128, kb * v], f8, name=f"{nm}_8_{c}"),
                             dram.tile([128, kb * v], f8, name=f"{nm}_r_{c}"))
                            for c in range(3)]
            zsb = const_p.tile([128, 4096], f8)
            nc.vector.memset(zsb[:], 0.0)

            def zero_vols(names):
                for nm in names:
                    kb, v = VOLS[nm]
                    tot = kb * v
                    for c in range(3):
                        for t8 in vols[nm][c]:
                            for c0 in range(0, tot, 4096):
                                n = min(4096, tot - c0)
                                nc.gpsimd.dma_start(t8[:, c0:c0 + n],
                                                    zsb[:, :n])
            zero_vols(["x3"])

            featsd = dram.tile([128, 64 * 3], f16)
            ag1out = dram.tile([N_CORES * 128, 64 * 3], f16,
                               addr_space="Shared")
            ar_in = dram.tile([128, 32 * 24], f32)
            ar_out = dram.tile([128, 32 * 24], f32, addr_space="Shared")

            # ================= phase A: conv1 + conv2 =================
            with tc.tile_pool(name="pA", bufs=1) as pA, \
                 tc.tile_pool(name="patch_p", bufs=4) as patch_p, \
                 tc.tile_pool(name="stfp", bufs=3) as stfp:
                NF = 6  # ring of padded frames; conv2 lags conv1 by 3
                X2L = NF * 3364
                x2p8 = pA.tile([128, X2L], f8)
                x2pr8 = pA.tile([128, X2L], f8)
                for t8, eng in ((x2p8, nc.scalar), (x2pr8, nc.gpsimd)):
                    for c0 in range(0, X2L, 4096):
                        n = min(4096, X2L - c0)
                        eng.dma_start(t8[:, c0:c0 + n], zsb[:, :n])
                x2p8v = x2p8[:].rearrange("p (d h w) -> p d h w",
                                          d=NF, h=58, w=58)
                x2pr8v = x2pr8[:].rearrange("p (d h w) -> p d h w",
                                            d=NF, h=58, w=58)
                w1_sb = pA.tile([108, 128], f16)
                nc.sync.dma_start(w1_sb[:], din["w1"][:])
                w2_sb = pA.tile([128, 10 * 2 * 128], f8)
                nc.sync.dma_start(w2_sb[:], din["w2dr"][:])
                w2v = w2_sb[:].rearrange("p (pr i m) -> p pr i m", pr=10,
                                         i=2)
                c2slots = _conv2_slot_pairs()

                # x3 frame staging (full padded 30x30 frame, borders zero)
                x3stg = [(stgp.tile([128, 900], f8, name=f"x3s8_{i}"),
                          stgp.tile([128, 900], f8, name=f"x3sr_{i}"))
                         for i in range(2)]
                for s8, sr in x3stg:
                    nc.vector.memset(s8[:], 0.0)
                    nc.gpsimd.memset(sr[:], 0.0)

                PXCLIP = SEGLEN * H0 * 56

                def conv1_half(clip, d, hh):
                    patch_sb = patch_p.tile([108, 7 * 448], f16, tag="patch")
                    nc.sync.dma_start(
                        patch_sb[:],
                        din["patches"][:, clip * PXCLIP + d * H0 * 56
                                       + hh * 3136:
                                       clip * PXCLIP + d * H0 * 56
                                       + hh * 3136 + 3136])
                    stF = stfp.tile([128, 7 * 448], f16, tag="stF")
                    for rg7 in range(7):
                        pt = ps.tile([128, 448], f32, tag="ps")
                        nc.tensor.matmul(
                            pt[:], w1_sb[:],
                            patch_sb[:, rg7 * 448:(rg7 + 1) * 448],
                            start=True, stop=True)
                        nc.scalar.activation(
                            stF[:, rg7 * 448:(rg7 + 1) * 448], pt[:],
                            AF.Relu, bias=bias_sb[:, 0:1], scale=float(BN))
                    # move upper 64 partitions down to base 0, then max in
                    # place (two-input engine ops need equal base partitions)
                    wp = stfp.tile([64, 7 * 448], f16, tag="wp")
                    nc.sync.dma_start(wp[:], stF[64:128, :])
                    nc.vector.tensor_tensor(wp[:], wp[:], stF[0:64, :],
                                            ALU.max)
                    wp4 = wp[:].rearrange("p (r i two w) -> p r i two w",
                                          r=7, i=4, two=2)
                    hp = stfp.tile([64, 28, 56], f16, tag="hp")
                    hp4 = hp[:].rearrange("p (r i) w -> p r i w", r=7)
                    nc.vector.tensor_tensor(hp4, wp4[:, :, :, 0],
                                            wp4[:, :, :, 1], ALU.max)
                    rows = slice(1 + hh * 28, 29 + hh * 28)
                    s1 = (clip * 17 + d + 1) % NF
                    d8 = x2p8v[0:64, s1, rows, 1:57]
                    if (d + hh) % 2 == 0:
                        nc.scalar.activation(d8, hp[:], AF.Copy)
                    else:
                        nc.vector.tensor_copy(d8, hp[:])
                    dr8 = x2pr8v[0:64, s1, rows, 1:57]
                    nc.vector.tensor_tensor(dr8, hp[:], d8, ALU.subtract)

                def conv2_rg2(clip, dd, rg2, stF2):
                    # two rg groups share one PSUM bank (row pitch 58 kept
                    # uniform: second group lands at offset 4*58=232)
                    pt = ps.tile([128, 462], f32, tag="ps")
                    for half in range(2):
                        rg = rg2 * 2 + half
                        def slot_off(s):
                            # ring-resolved flat offset of a conv2 slot
                            kd, kh, hf = s
                            return (((clip * 17 + dd + kd) % NF) * 3364
                                    + kh * 58 + hf + (rg * 4) * 58)
                        i = 0
                        for x2t in (x2p8, x2pr8):
                            for pr in range(10):
                                sa, sb = c2slots[pr]
                                oa = slot_off(sa)
                                delta = 0 if sb is None \
                                    else slot_off(sb) - oa
                                base = x2t[:, oa:oa + 1]
                                rhs = _ap_dims(base, [[delta, 2], [1, 230]])
                                nc.tensor.matmul(
                                    pt[:, half * 232:half * 232 + 230],
                                    w2v[:, pr], rhs,
                                    start=(i == 0), stop=(i == 19),
                                    perf_mode=DR)
                                i += 1
                    src = _ap_dims(pt[:, 0:1], [[58, 8], [1, 56]])
                    nc.scalar.activation(stF2[:, rg2 * 8:(rg2 + 1) * 8, :],
                                         src, AF.Relu,
                                         bias=bias_sb[:, 1:2],
                                         scale=float(BN))

                def conv2_frame_tail(clip, dd, stF2, dstage):
                    wpF = pool1.tile([128, 56, 28], f16, tag="wp2")
                    nc.vector.tensor_tensor(wpF[:], stF2[:, :, 0::2],
                                            stF2[:, :, 1::2], ALU.max)
                    hpF = dstp.tile([128, 28, 28], f16, tag="hp2")
                    nc.vector.tensor_tensor(hpF[:], wpF[:, 0::2, :],
                                            wpF[:, 1::2, :], ALU.max)
                    if dd % 2 == 0:
                        dstage["f"] = hpF
                        return
                    dpF = pool2.tile([128, 28, 28], f16, tag="dp")
                    nc.vector.tensor_tensor(dpF[:], dstage["f"][:], hpF[:],
                                            ALU.max)
                    s8, sr = x3stg[(dd // 2) % 2]
                    s8v = s8[:].rearrange("p (h w) -> p h w", h=30)
                    srv = sr[:].rearrange("p (h w) -> p h w", h=30)
                    t8 = s8v[:, 1:29, 1:29]
                    nc.scalar.activation(t8, dpF[:], AF.Copy)
                    nc.vector.tensor_tensor(srv[:, 1:29, 1:29], dpF[:], t8,
                                            ALU.subtract)

                LAG = 3
                dstage = {}
                for g in range(48 + LAG):
                    if g < 48:
                        clip, d = divmod(g, 16)
                        for hh in range(2):
                            conv1_half(clip, d, hh)
                        s1 = (clip * 17 + d + 1) % NF
                        # shifted upper-half copies (tap kw+2 packing)
                        nc.sync.dma_start(x2p8v[64:128, s1, 0:58, 0:56],
                                          x2p8v[0:64, s1, 0:58, 2:58])
                        nc.sync.dma_start(x2pr8v[64:128, s1, 0:58, 0:56],
                                          x2pr8v[0:64, s1, 0:58, 2:58])
                    if g >= LAG:
                        clip, dd = divmod(g - LAG, 16)
                        if dd == 13:
                            # re-zero the shared pad slot between clips
                            sp = ((clip + 1) * 17) % NF
                            nc.gpsimd.dma_start(
                                x2p8[:, sp * 3364:(sp + 1) * 3364],
                                zsb[:, :3364])
                            nc.gpsimd.dma_start(
                                x2pr8[:, sp * 3364:(sp + 1) * 3364],
                                zsb[:, :3364])
                        stF2 = stfp.tile([128, 56, 56], f16, tag="stF2")
                        for rg2 in range(7):
                            conv2_rg2(clip, dd, rg2, stF2)
                        conv2_frame_tail(clip, dd, stF2, dstage)
                        if dd % 2 == 1:
                            dout = dd // 2
                            s8, sr = x3stg[dout % 2]
                            v8, vr = vols["x3"][clip]
                            nc.sync.dma_start(
                                v8[:, (dout + 1) * 900:(dout + 2) * 900],
                                s8[:])
                            nc.sync.dma_start(
                                vr[:, (dout + 1) * 900:(dout + 2) * 900],
                                sr[:])

            zero_vols(["x3b", "x4", "x4b", "x5", "x5b"])

            # ================= phase B: conv3a .. conv5b =================
            def conv_layer(wname, invols, outvols):
                KB, MB, D, Hs, Ws, pool, bias_col, scale = CONVS[wname]
                PD, PH, PW = D + 2, Hs + 2, Ws + 2
                V = PD * PH * PW
                RG = 14 if Hs == 14 else 7
                n_rg = Hs // RG
                N = (RG - 1) * PW + Ws
                taps = _tap_list(KB)
                offs = [((kb * PD + kd) * PH + kh) * PW + kw
                        for kb, kd, kh, kw in taps]
                ppairs = _dr_pairs(KB, PD, PH, PW)
                npr = len(ppairs)
                pairs = []
                for ta, tb in ppairs:
                    a = offs[ta]
                    b = offs[ta] if tb is None else offs[tb]
                    pairs.append((a, b - a))
                if pool == "222":
                    PHo, PWo = Hs // 2 + 2, Ws // 2 + 2
                    wps = [stgp.tile([128, Hs * (Ws // 2)], f16,
                                     name=f"{wname}_wps_{i}")
                           for i in range(2)]
                elif pool is None:
                    PHo, PWo = PH, PW
                if pool != "5":
                    stg = [(stgp.tile([128, PHo * PWo], f8,
                                      name=f"{wname}_s8_{i}"),
                            stgp.tile([128, PHo * PWo], f8,
                                      name=f"{wname}_sr_{i}"))
                           for i in range(3)]
                    for s8, sr in stg:
                        nc.vector.memset(s8[:], 0.0)
                        nc.gpsimd.memset(sr[:], 0.0)
                si = [0]
                WCOL = npr * 2 * 128
                for mb in range(MB):
                    wt = wpool.tile([128, WCOL], f8, tag="w")
                    nc.sync.dma_start(wt[:],
                                      din[wname][:, mb * WCOL:(mb + 1) * WCOL])
                    wtv = wt[:].rearrange("p (pr i m) -> p pr i m",
                                          pr=npr, i=2)
                    for clip in range(3):
                        x8t = xpool.tile([128, KB * V], f8, tag="x8")
                        nc.sync.dma_start(x8t[:], invols[clip][0][:])
                        r8t = xpool.tile([128, KB * V], f8, tag="r8")
                        nc.sync.dma_start(r8t[:], invols[clip][1][:])
                        dstage = {}
                        for d in range(D):
                            for rg in range(n_rg):
                                wbase = d * PH * PW + rg * RG * PW
                                pt = ps.tile([128, N], f32, tag="ps")
                                i = 0
                                nmm = 2 * npr
                                for xt in (x8t, r8t):
                                    for a, delta in pairs:
                                        base = xt[:, wbase + a:wbase + a + 1]
                                        rhs = _ap_dims(base,
                                                       [[delta, 2], [1, N]])
                                        nc.tensor.matmul(
                                            pt[:], wtv[:, i % npr], rhs,
                                            start=(i == 0),
                                            stop=(i == nmm - 1),
                                            perf_mode=DR)
                                        i += 1
                                y16 = stp.tile([128, RG, Ws], f16, tag="y")
                                src = _ap_dims(pt[:, 0:1],
                                               [[PW, RG], [1, Ws]])
                                nc.scalar.activation(
                                    y16[:], src, AF.Relu,
                                    bias=bias_sb[:, bias_col + mb:
                                                 bias_col + mb + 1],
                                    scale=scale)
                                if pool is None:
                                    s8, sr = stg[si[0] % 3]
                                    s8v = s8[:].rearrange(
                                        "p (h w) -> p h w", h=PH)
                                    srv = sr[:].rearrange(
                                        "p (h w) -> p h w", h=PH)
                                    rr = slice(rg * RG + 1, rg * RG + 1 + RG)
                                    t8 = s8v[:, rr, 1:1 + Ws]
                                    ew().tensor_copy(t8, y16[:])
                                    ew().tensor_tensor(srv[:, rr, 1:1 + Ws],
                                                       y16[:], t8,
                                                       ALU.subtract)
                                elif pool == "222":
                                    wv = wps[d % 2][:].rearrange(
                                        "p (h w) -> p h w", h=Hs)
                                    ew().tensor_tensor(
                                        wv[:, rg * RG:rg * RG + RG, :],
                                        y16[:, :, 0::2], y16[:, :, 1::2],
                                        ALU.max)
                                else:  # conv5b: y16 is [128, 7, 7]
                                    if d % 2 == 0:
                                        dstage["f"] = y16
                                    else:
                                        dmx = pool1.tile([128, 7, 7], f16,
                                                         tag="dmx")
                                        ew().tensor_tensor(dmx[:],
                                                           dstage["f"][:],
                                                           y16[:], ALU.max)
                                        wp5 = pool2.tile([128, 7, 4], f16,
                                                         tag="wp5")
                                        nc.vector.tensor_copy(
                                            wp5[:, :, 0:1], dmx[:, :, 0:1])
                                        nc.vector.tensor_tensor(
                                            wp5[:, :, 1:4], dmx[:, :, 1:6:2],
                                            dmx[:, :, 2:7:2], ALU.max)
                                        hp5 = pool2.tile([128, 4, 4], f16,
                                                         tag="hp5")
                                        nc.vector.tensor_copy(
                                            hp5[:, 0:1, :], wp5[:, 0:1, :])
                                        nc.vector.tensor_tensor(
                                            hp5[:, 1:4, :], wp5[:, 1:6:2, :],
                                            wp5[:, 2:7:2, :], ALU.max)
                                        fv = featsd[:].rearrange(
                                            "p (m h w c) -> p m h w c",
                                            m=4, h=4, w=4)
                                        nc.sync.dma_start(fv[:, mb, :, :,
                                                             clip],
                                                          hp5[:])
                            # ---- end rg loop: frame-level epilogue ----
                            if pool is None:
                                s8, sr = stg[si[0] % 3]
                                si[0] += 1
                                v8, vr = outvols[clip]
                                fo = (mb * PD + d + 1) * PH * PW
                                nc.sync.dma_start(
                                    v8[:, fo:fo + PH * PW], s8[:])
                                nc.sync.dma_start(
                                    vr[:, fo:fo + PH * PW], sr[:])
                            elif pool == "222":
                                wv = wps[d % 2][:].rearrange(
                                    "p (h w) -> p h w", h=Hs)
                                hp = pool2.tile([128, Hs // 2, Ws // 2],
                                                f16, tag="hpL")
                                ew().tensor_tensor(hp[:], wv[:, 0::2, :],
                                                   wv[:, 1::2, :], ALU.max)
                                if d % 2 == 0:
                                    dstage["f"] = hp
                                else:
                                    dp = pool2.tile(
                                        [128, Hs // 2, Ws // 2], f16,
                                        tag="dpL")
                                    ew().tensor_tensor(dp[:],
                                                       dstage["f"][:],
                                                       hp[:], ALU.max)
                                    s8, sr = stg[si[0] % 3]
                                    si[0] += 1
                                    s8v = s8[:].rearrange(
                                        "p (h w) -> p h w", h=PHo)
                                    srv = sr[:].rearrange(
                                        "p (h w) -> p h w", h=PHo)
                                    hh = slice(1, 1 + Hs // 2)
                                    wwc = slice(1, 1 + Ws // 2)
                                    t8 = s8v[:, hh, wwc]
                                    ew().tensor_copy(t8, dp[:])
                                    ew().tensor_tensor(srv[:, hh, wwc],
                                                       dp[:], t8,
                                                       ALU.subtract)
                                    v8, vr = outvols[clip]
                                    dout = d // 2
                                    fo = (mb * (D // 2 + 2) + dout + 1) \
                                        * PHo * PWo
                                    nc.sync.dma_start(
                                        v8[:, fo:fo + PHo * PWo], s8[:])
                                    nc.sync.dma_start(
                                        vr[:, fo:fo + PHo * PWo], sr[:])

            fcp = ctx.enter_context(tc.tile_pool(name="fcp", bufs=1))
            with tc.tile_pool(name="wpool", bufs=3) as wpool, \
                 tc.tile_pool(name="xpool", bufs=2) as xpool:
                conv_layer("w3a", vols["x3"], vols["x3b"])
                conv_layer("w3b", vols["x3b"], vols["x4"])
                conv_layer("w4a", vols["x4"], vols["x4b"])
                conv_layer("w4b", vols["x4b"], vols["x5"])
            with tc.tile_pool(name="wpool", bufs=4) as wpool, \
                 tc.tile_pool(name="xpool", bufs=2) as xpool:
                f6w_sb = fcp.tile([128, 4 * 64 * 128], f16)
                for q in range(4):
                    nc.scalar.dma_start(
                        f6w_sb[:, q * 8192:(q + 1) * 8192],
                        din["fc6w"][:, q * 8192:(q + 1) * 8192])
                f7w_sb = fcp.tile([128, 32 * 4 * 128], f16)
                for q in range(4):
                    nc.scalar.dma_start(
                        f7w_sb[:, q * 4096:(q + 1) * 4096],
                        din["fc7w"][:, q * 4096:(q + 1) * 4096])
                conv_layer("w5a", vols["x5"], vols["x5b"])
                conv_layer("w5b", vols["x5b"], None)

            # ================= phase C: FC + gram + sinkhorn =================
            # One AllGather (conv features) + fc6 sharded over 512 outputs;
            # fc7 is linear in the fc6 blocks, so each core computes fc7
            # partials from its local fc6 slice and one AllReduce replaces
            # the other two gathers.
            nc.gpsimd.collective_compute(
                "AllGather", ALU.bypass,
                replica_groups=[list(range(N_CORES))],
                ins=[featsd.opt()], outs=[ag1out.opt()])

            if True:
                rhs6 = fcp.tile([128, 64, 24], f16)
                agv = ag1out[:].rearrange("(r p) k -> r p k", r=8)
                r6v = rhs6[:].rearrange("p kb (r cl) -> p kb r cl", r=8)
                for r in range(8):
                    nc.sync.dma_start(r6v[:, :, r], agv[r])
                a6 = fcp.tile([128, 4, 24], f16)
                for mb in range(4):
                    pt = ps.tile([128, 24], f32, tag="ps")
                    for kb in range(64):
                        nc.tensor.matmul(
                            pt[:], f6w_sb[:, (mb * 64 + kb) * 128:
                                          (mb * 64 + kb + 1) * 128],
                            rhs6[:, kb], start=(kb == 0), stop=(kb == 63))
                    nc.scalar.activation(a6[:, mb], pt[:], AF.Relu,
                                         bias=bias_sb[:, 22 + mb:23 + mb],
                                         scale=float(BN))
                arst = fcp.tile([128, 32, 24], f32)
                for omb in range(32):
                    pt = ps.tile([128, 24], f32, tag="ps")
                    for kbl in range(4):
                        nc.tensor.matmul(
                            pt[:], f7w_sb[:, (omb * 4 + kbl) * 128:
                                          (omb * 4 + kbl + 1) * 128],
                            a6[:, kbl], start=(kbl == 0), stop=(kbl == 3))
                    if omb % 2:
                        nc.vector.tensor_copy(arst[:, omb], pt[:])
                    else:
                        nc.scalar.activation(arst[:, omb], pt[:], AF.Copy)
                nc.sync.dma_start(ar_in[:], arst[:])
                nc.gpsimd.collective_compute(
                    "AllReduce", ALU.add,
                    replica_groups=[list(range(N_CORES))],
                    ins=[ar_in.opt()], outs=[ar_out.opt()])

                arsb = arst
                nc.sync.dma_start(arsb[:], ar_out[:])
                fr = fcp.tile([128, 32, 24], f16)
                for omb in range(32):
                    nc.scalar.activation(fr[:, omb], arsb[:, omb], AF.Relu,
                                         bias=bias_sb[:, 26 + omb:27 + omb],
                                         scale=float(BN))
                gps = ps.tile([24, 24], f32, tag="ps")
                for kb in range(32):
                    nc.tensor.matmul(gps[:], fr[:, kb], fr[:, kb],
                                     start=(kb == 0), stop=(kb == 31))

                g_sb = sk.tile([24, 24], f32)
                nc.vector.tensor_copy(g_sb[:], gps[:])
                gdram = dram.tile([24, 24], f32)
                nc.sync.dma_start(gdram[:], g_sb[:])
                gflat = gdram[:].rearrange("a b -> (a b)")
                dg = sk.tile([1, 24], f32)
                nc.sync.dma_start(dg[:], gflat[None, ::25])
                sq = sk.tile([1, 24], f32)
                nc.scalar.activation(sq[:], dg[:], AF.Sqrt)
                nc.vector.tensor_scalar_add(sq[:], sq[:], 1e-8)
                inv = sk.tile([1, 24], f32)
                nc.vector.reciprocal(inv[:], sq[:])
                invd = dram.tile([1, 24], f32)
                nc.sync.dma_start(invd[:], inv[:])
                inv_col = sk.tile([24, 1], f32)
                nc.sync.dma_start(inv_col[:],
                                  invd[:].rearrange("a b -> (a b)")[:, None])
                t1 = sk.tile([24, 24], f32)
                nc.vector.tensor_scalar_mul(t1[:], g_sb[:], inv_col[:])
                eye_sb = sk.tile([24, 24], f32)
                nc.sync.dma_start(eye_sb[:], din["eye24"][:])
                tps = ps.tile([24, 24], f32, tag="ps")
                nc.tensor.transpose(tps[:], t1[:], eye_sb[:])
                t2 = sk.tile([24, 24], f32)
                nc.vector.tensor_copy(t2[:], tps[:])
                cos_sb = sk.tile([24, 24], f32)
                nc.vector.tensor_scalar_mul(cos_sb[:], t2[:], inv_col[:])
                cosd = dram.tile([24, 24], f32)
                nc.sync.dma_start(cosd[:], cos_sb[:])

                cos_ij = sk.tile([9, 4, 4], f32)
                for qv in range(3):
                    for sv in range(3):
                        p = qv * 3 + sv
                        src = cosd[:][None, 12 + qv * 4:12 + qv * 4 + 4,
                                      sv * 4:sv * 4 + 4]
                        nc.sync.dma_start(cos_ij[p:p + 1], src)

                bmat_sb = sk.tile([9, 4, 4], f32)
                nc.sync.dma_start(
                    bmat_sb[:],
                    din["bmat"][:].rearrange("p (i j) -> p i j", i=4))
                arg = sk.tile([9, 4, 4], f32)
                nc.vector.tensor_scalar_mul(arg[:], cos_ij[:], float(REG))
                nc.vector.tensor_tensor(arg[:], arg[:], bmat_sb[:], ALU.add)
                kt = sk.tile([9, 4, 4], f32)
                nc.scalar.activation(kt[:], arg[:], AF.Exp)
                ktT = sk.tile([9, 4, 4], f32)
                nc.vector.tensor_copy(ktT[:],
                                      kt[:].rearrange("p i j -> p j i"))
                sem = sk.tile([9, 4, 4], f32)
                nc.vector.tensor_scalar(sem[:], cos_ij[:], -1.0, 1.0,
                                        ALU.mult, ALU.add)
                msem = sk.tile([9, 4, 4], f32)
                nc.vector.tensor_tensor(msem[:], kt[:], sem[:], ALU.mult)

                u = sk.tile([9, 4], f32)
                nc.vector.memset(u[:], 0.25)
                prod = sk.tile([9, 4, 4], f32)
                s = sk.tile([9, 4], f32)
                v = sk.tile([9, 4], f32)
                EPS4 = 4e-9
                for it in range(SINK_ITERS + 1):
                    nc.vector.tensor_tensor(
                        prod[:], ktT[:],
                        u[:, None, :].broadcast_to([9, 4, 4]), ALU.mult)
                    nc.vector.reduce_sum(s[:, :, None], prod[:],
                                         axis=mybir.AxisListType.X)
                    nc.vector.tensor_scalar_add(s[:], s[:], EPS4)
                    nc.vector.reciprocal(v[:], s[:])
                    if it == SINK_ITERS:
                        break
                    nc.vector.tensor_tensor(
                        prod[:], kt[:],
                        v[:, None, :].broadcast_to([9, 4, 4]), ALU.mult)
                    nc.vector.reduce_sum(s[:, :, None], prod[:],
                                         axis=mybir.AxisListType.X)
                    nc.vector.tensor_scalar_add(s[:], s[:], EPS4)
                    nc.vector.reciprocal(u[:], s[:])

                ta = sk.tile([9, 4, 4], f32)
                nc.vector.tensor_tensor(
                    ta[:], msem[:],
                    u[:, :, None].broadcast_to([9, 4, 4]), ALU.mult)
                nc.vector.tensor_tensor(
                    ta[:], ta[:],
                    v[:, None, :].broadcast_to([9, 4, 4]), ALU.mult)
                t9s = sk.tile([9, 1], f32)
                nc.vector.reduce_sum(t9s[:, :, None], ta[:],
                                     axis=mybir.AxisListType.XY)
                o9 = sk.tile([9, 1], f32)
                nc.scalar.mul(o9[:], t9s[:], -0.25)
                nc.sync.dma_start(out_d[:], o9[:])

    nc.compile()
    return nc


def kernel(**inputs):
    from concourse.bass_utils import run_bass_kernel_spmd
    if "nc" not in _BUILD_CACHE:
        _BUILD_CACHE["nc"] = _build()
    nc = _BUILD_CACHE["nc"]
    in_maps = _prep_inputs(inputs)
    res = run_bass_kernel_spmd(nc, in_maps, core_ids=list(range(N_CORES)))
    return res.results[0]["out"].reshape(3, 3).astype(np.float32)


# revision 40
# speedup vs baseline: 1.0309x; 1.0309x over previous
"""Trainium2 Bass kernel for nn_C3D_15470472200649.

C3D video encoder (8 conv3d layers + fc6/fc7) + pairwise cosine + Sinkhorn OT.
Sharding: data-parallel over the 24 clips (3 per core) for the encoder;
fc6/fc7 sharded over output features (512/core); features exchanged with
AllGather; the tiny OT stage is replicated on every core.

conv1 runs in fp16 with host-side im2col packing two adjacent output columns
into the 128 output partitions (K=108 = 3cin x 3x3 taps x 4-wide windows).
conv2..conv5b run as fp8(e4m3) DoubleRow matmuls: each instruction contracts
TWO conv taps (constant address delta on the pair axis) at 0.5 cycles/row.
Taps are paired so the pair-axis byte delta is EVEN (odd deltas fault on
hardware). Accuracy is held by an activation residual split: activations are
stored as x8 + r8 (both fp8, r8 = fp16 value minus x8), and every conv
accumulates w8*x8 + w8*r8 into fp32 PSUM; weights are single fp8 (their
quantization error is shared between query and support branches and largely
cancels in the cosine). Inter-layer volumes are zero-padded fp8 pairs;
windows span padded rows flat-contiguously so the DoubleRow moving operand
stays 3-dim. conv1/conv2 are software-pipelined through a 6-frame SBUF ring;
the tail uses one AllGather (conv features) + one AllReduce (fc7 partials,
fc7 being linear in the sharded fc6 blocks); the tiny Sinkhorn runs
replicated on every core.
"""

import dataclasses
import math
import numpy as np
import ml_dtypes

N_CORES = 8
SEGLEN, CIN, H0, W0 = 16, 3, 112, 112
REG, COST_ALPHA = 7.0, 0.4
SINK_ITERS = 12          # converges exactly by ~10; reference runs 100
BN = np.float32(1.0 / np.sqrt(1.0 + 1e-5))
F16 = np.float16
F8 = ml_dtypes.float8_e4m3


def _pos_cost():
    t = np.arange(4, dtype=np.float32) / 4.0
    d2 = (t[:, None] - t[None, :]) ** 2
    return np.exp(-(1.0 / (d2 + 1.0))).astype(np.float32)


# ---------------- host-side preparation ----------------

def _tap_list(KB):
    taps = []
    for kb in range(KB):
        for kd in range(3):
            for kh in range(3):
                for kw in range(3):
                    taps.append((kb, kd, kh, kw))
    return taps


def _dr_pairs(KB, PD, PH, PW):
    """Pair taps so the rhs pair-axis delta is EVEN (odd deltas fault on HW).

    Returns list of (tap_a, tap_b_or_None) index pairs into _tap_list(KB).
    """
    taps = _tap_list(KB)
    V = PD * PH * PW
    off = [((kb * PD + kd) * PH + kh) * PW + kw for kb, kd, kh, kw in taps]
    ev = [i for i in range(len(taps)) if off[i] % 2 == 0]
    od = [i for i in range(len(taps)) if off[i] % 2 == 1]
    pairs = []
    for lst in (ev, od):
        for j in range(0, len(lst) - 1, 2):
            pairs.append((lst[j], lst[j + 1]))
        if len(lst) % 2:
            pairs.append((lst[-1], None))
    return pairs


def _conv_w_dr(w, KB, MB, dims):
    """w (Cout, Cin, 3,3,3) -> [128, MB*npairs*2*128] f8 DoubleRow pairs."""
    Cout, Cin = w.shape[:2]
    wm = w.transpose(2, 3, 4, 1, 0).reshape(3, 3, 3, Cin, Cout)
    taps = _tap_list(KB)
    pairs = _dr_pairs(KB, *dims)
    out = np.zeros((128, MB, len(pairs), 2, 128), np.float32)
    PK = Cin // KB
    PM = Cout // MB
    for mb in range(MB):
        for pi, (ta, tb) in enumerate(pairs):
            for i, ti in enumerate((ta, tb)):
                if ti is None:
                    continue  # zero pad slot
                kb, kd, kh, kw = taps[ti]
                out[:PK, mb, pi, i, :PM] = wm[kd, kh, kw,
                                              kb * PK:(kb + 1) * PK,
                                              mb * PM:(mb + 1) * PM]
    return out.reshape(128, -1).astype(F8)


def _conv2_slot_pairs():
    """conv2 DR slot pairing: full slots (kw0 lower/kw2 upper, even offset)
    pair together; half slots (kw1, odd offset) pair together."""
    full = [(kd, kh, 0) for kd in range(3) for kh in range(3)]
    half = [(kd, kh, 1) for kd in range(3) for kh in range(3)]
    pairs = []
    for lst in (full, half):
        for j in range(0, 8, 2):
            pairs.append((lst[j], lst[j + 1]))
        pairs.append((lst[8], None))
    return pairs


def _conv2_w_dr(w2):
    """conv2 (128, 64, 3,3,3) -> [128, 9*2*128] f8.

    9 DR pairs, one per (kd, kh): slot0 = partition-packed (kw0 lower 64,
    kw2 upper 64); slot1 = kw1 lower, zero upper. Pair delta in rhs = +1.
    """
    wm = w2.transpose(2, 3, 4, 1, 0)  # (kd, kh, kw, 64, 128)
    out = np.zeros((128, 10, 2, 128), np.float32)
    for pi, (sa, sb) in enumerate(_conv2_slot_pairs()):
        for i, s in enumerate((sa, sb)):
            if s is None:
                continue
            kd, kh, half = s
            if half:
                out[:64, pi, i, :] = wm[kd, kh, 1]
            else:
                out[:64, pi, i, :] = wm[kd, kh, 0]
                out[64:, pi, i, :] = wm[kd, kh, 2]
    return out.reshape(128, -1).astype(F8)


def _conv1_w_packed(w1):
    """(64, 3, 3,3,3) -> [108, 128] f16; col = pos*64 + ch; row =
    ((c*3+kd)*3+kh)*4 + kw4; value = w1[ch,c,kd,kh,kw4-pos] (0 outside)."""
    out = np.zeros((108, 128), np.float32)
    for c in range(CIN):
        for kd in range(3):
            for kh in range(3):
                for kw4 in range(4):
                    r = ((c * 3 + kd) * 3 + kh) * 4 + kw4
                    for pos in range(2):
                        kw = kw4 - pos
                        if 0 <= kw <= 2:
                            out[r, pos * 64:pos * 64 + 64] = w1[:, c, kd, kh, kw]
    return out.astype(F16)


def _im2col_clip_packed(clip):
    """clip (3, 16, 112, 112) -> [108, 16*112*56] f16, cols (d, h, w')."""
    xp = np.zeros((CIN, SEGLEN + 2, H0 + 2, W0 + 2), np.float32)
    xp[:, 1:-1, 1:-1, 1:-1] = clip
    out = np.empty((108, SEGLEN * H0 * 56), F16)
    for c in range(CIN):
        for kd in range(3):
            for kh in range(3):
                for kw4 in range(4):
                    r = ((c * 3 + kd) * 3 + kh) * 4 + kw4
                    sl = xp[c, kd:kd + SEGLEN, kh:kh + H0, kw4:kw4 + 111:2]
                    out[r] = sl.reshape(-1).astype(F16)
    return out


def _prep_inputs(inputs):
    sup = np.asarray(inputs["support_set"], np.float32)
    qry = np.asarray(inputs["query_set"], np.float32)
    sp = np.swapaxes(sup, 2, 3).reshape(-1, CIN, SEGLEN, H0, W0)
    qr = np.swapaxes(qry, 2, 3).reshape(-1, CIN, SEGLEN, H0, W0)
    clips = np.concatenate([sp, qr], 0)  # 0-11 support, 12-23 query

    w1p = _conv1_w_packed(np.asarray(inputs["conv1_w"], np.float32))
    w2dr = _conv2_w_dr(np.asarray(inputs["conv2_w"], np.float32))
    w3a = _conv_w_dr(np.asarray(inputs["conv3a_w"], np.float32), 1, 2,
                     (10, 30, 30))
    w3b = _conv_w_dr(np.asarray(inputs["conv3b_w"], np.float32), 2, 2,
                     (10, 30, 30))
    w4a = _conv_w_dr(np.asarray(inputs["conv4a_w"], np.float32), 2, 4,
                     (6, 16, 16))
    w4b = _conv_w_dr(np.asarray(inputs["conv4b_w"], np.float32), 4, 4,
                     (6, 16, 16))
    w5a = _conv_w_dr(np.asarray(inputs["conv5a_w"], np.float32), 4, 4,
                     (4, 9, 9))
    w5b = _conv_w_dr(np.asarray(inputs["conv5b_w"], np.float32), 4, 4,
                     (4, 9, 9))
    fc6w = np.asarray(inputs["fc6_w"], np.float32)
    fc7w = np.asarray(inputs["fc7_w"], np.float32)

    def bc(b, scale, blocks):
        cols = np.zeros((128, blocks), np.float32)
        b = np.asarray(b, np.float32) * scale
        n = b.size // blocks
        for m in range(blocks):
            cols[:n, m] = b[m * n:(m + 1) * n]
        return cols

    b1 = np.asarray(inputs["conv1_b"], np.float32) * BN
    b1col = np.concatenate([b1, b1])[:, None]  # replicated for 2-pos packing

    pos = _pos_cost()
    bmat = np.zeros((9, 16), np.float32)
    bmat[:] = (math.log(4.0) - REG - REG * COST_ALPHA * pos).reshape(-1)[None]
    eye24 = np.eye(24, dtype=np.float32)

    def fc6_lhsT(w_slice):
        # feature f=(mbk*128+p)*16+hw <-> kb=(mbk,h,w), partition p
        a = w_slice.reshape(4, 128, 4, 128, 16)      # (omb, o, mbk, p, hw)
        a = a.transpose(3, 0, 2, 4, 1)               # (p, omb, mbk, hw, o)
        return a.reshape(128, 4 * 64 * 128).astype(F16)

    def fc7_lhsT(w_cols):
        # w_cols (4096, 512): local K blocks kbl of 128
        a = w_cols.reshape(32, 128, 4, 128)          # (omb, o, kbl, p)
        a = a.transpose(3, 0, 2, 1)                  # (p, omb, kbl, o)
        return a.reshape(128, 32 * 4 * 128).astype(F16)

    in_maps = []
    for core in range(N_CORES):
        patches = np.concatenate(
            [_im2col_clip_packed(clips[core * 3 + c]) for c in range(3)],
            axis=1)
        r0, r1 = core * 512, (core + 1) * 512
        bias = np.concatenate([
            b1col, bc(inputs["conv2_b"], BN, 1),
            bc(inputs["conv3a_b"], 1.0, 2), bc(inputs["conv3b_b"], BN, 2),
            bc(inputs["conv4a_b"], 1.0, 4), bc(inputs["conv4b_b"], BN, 4),
            bc(inputs["conv5a_b"], 1.0, 4), bc(inputs["conv5b_b"], BN, 4),
            bc(np.asarray(inputs["fc6_b"])[r0:r1], BN, 4),
            bc(np.asarray(inputs["fc7_b"]), BN, 32),
        ], axis=1)
        in_maps.append({
            "patches": patches,
            "w1": w1p, "w2dr": w2dr,
            "w3a": w3a, "w3b": w3b, "w4a": w4a, "w4b": w4b,
            "w5a": w5a, "w5b": w5b,
            "fc6w": fc6_lhsT(fc6w[r0:r1]),
            "fc7w": fc7_lhsT(fc7w[:, r0:r1]),
            "bias": bias, "bmat": bmat, "eye24": eye24,
        })
    return in_maps


# ---------------- device program ----------------

_BUILD_CACHE = {}


def _ap_dims(ap_obj, dims):
    """Replace the free dims of a 1-elem AP base with explicit [stride,size]."""
    return dataclasses.replace(ap_obj, ap=[list(ap_obj.ap[0])] + dims)


def _build():
    import contextlib
    import concourse.bass as bass  # noqa: F401
    import concourse.tile as tile
    from concourse import bacc, mybir

    f16 = mybir.dt.float16
    f32 = mybir.dt.float32
    f8 = mybir.dt.float8e4
    AF = mybir.ActivationFunctionType
    ALU = mybir.AluOpType
    DR = mybir.MatmulPerfMode.DoubleRow

    nc = bacc.Bacc("TRN2", target_bir_lowering=False, debug=False,
                   num_devices=N_CORES)

    din = {}
    din["patches"] = nc.dram_tensor("patches", [108, 3 * SEGLEN * H0 * 56],
                                    f16, kind="ExternalInput")
    din["w1"] = nc.dram_tensor("w1", [108, 128], f16, kind="ExternalInput")
    din["w2dr"] = nc.dram_tensor("w2dr", [128, 10 * 2 * 128], f8,
                                 kind="ExternalInput")
    CONVS = {
        # name: (KB, MB, D, Hs, Ws, pool, bias_col, scale)
        "w3a": (1, 2, 8, 28, 28, None, 2, 1.0),
        "w3b": (2, 2, 8, 28, 28, "222", 4, float(BN)),
        "w4a": (2, 4, 4, 14, 14, None, 6, 1.0),
        "w4b": (4, 4, 4, 14, 14, "222", 10, float(BN)),
        "w5a": (4, 4, 2, 7, 7, None, 14, 1.0),
        "w5b": (4, 4, 2, 7, 7, "5", 18, float(BN)),
    }
    for nm, (kb, mb) in [(k, (v[0], v[1])) for k, v in CONVS.items()]:
        npr = (kb * 27 + 1) // 2
        din[nm] = nc.dram_tensor(nm, [128, mb * npr * 2 * 128], f8,
                                 kind="ExternalInput")
    din["fc6w"] = nc.dram_tensor("fc6w", [128, 4 * 64 * 128], f16,
                                 kind="ExternalInput")
    din["fc7w"] = nc.dram_tensor("fc7w", [128, 4 * 32 * 128], f16,
                                 kind="ExternalInput")
    din["bias"] = nc.dram_tensor("bias", [128, 58], f32, kind="ExternalInput")
    din["bmat"] = nc.dram_tensor("bmat", [9, 16], f32, kind="ExternalInput")
    din["eye24"] = nc.dram_tensor("eye24", [24, 24], f32, kind="ExternalInput")
    out_d = nc.dram_tensor("out", [9, 1], f32, kind="ExternalOutput")

    with tile.TileContext(nc) as tc:
        ctx = contextlib.ExitStack()
        with ctx:
            dram = ctx.enter_context(tc.tile_pool(name="dram", bufs=1,
                                                  space="DRAM"))
            ps = ctx.enter_context(tc.tile_pool(name="ps", bufs=8,
                                                space="PSUM"))
            const_p = ctx.enter_context(tc.tile_pool(name="const", bufs=1))
            stp = ctx.enter_context(tc.tile_pool(name="stp", bufs=4))
            pool1 = ctx.enter_context(tc.tile_pool(name="pool1", bufs=2))
            pool2 = ctx.enter_context(tc.tile_pool(name="pool2", bufs=4))
            dstp = ctx.enter_context(tc.tile_pool(name="dstp", bufs=3))
            stgp = ctx.enter_context(tc.tile_pool(name="stgp", bufs=1))
            sk = ctx.enter_context(tc.tile_pool(name="sk", bufs=1))

            bias_sb = const_p.tile([128, 58], f32)
            nc.sync.dma_start(bias_sb[:], din["bias"][:])

            # elementwise engine alternation (DVE : gpsimd = 2 : 1)
            ew_state = [0]

            def ew():
                ew_state[0] += 1
                return nc.vector

            # DRAM inter-layer volumes (padded fp8 x8/r8 pairs)
            VOLS = {
                "x3": (1, 10 * 30 * 30), "x3b": (2, 10 * 30 * 30),
                "x4": (2, 6 * 16 * 16), "x4b": (4, 6 * 16 * 16),
                "x5": (4, 4 * 9 * 9), "x5b": (4, 4 * 9 * 9),
            }
            vols = {}
            for nm, (kb, v) in VOLS.items():
                vols[nm] = [(dram.tile([128, kb * v], f8, name=f"{nm}_8_{c}"),
                             dram.tile([128, kb * v], f8, name=f"{nm}_r_{c}"))
                            for c in range(3)]
            zsb = const_p.tile([128, 4096], f8)
            nc.vector.memset(zsb[:], 0.0)

            def zero_vols(names):
                for nm in names:
                    kb, v = VOLS[nm]
                    tot = kb * v
                    for c in range(3):
                        for t8 in vols[nm][c]:
                            for c0 in range(0, tot, 4096):
                                n = min(4096, tot - c0)
                                nc.gpsimd.dma_start(t8[:, c0:c0 + n],
                                                    zsb[:, :n])
            zero_vols(["x3"])

            featsd = dram.tile([128, 64 * 3], f16)
            ag1out = dram.tile([N_CORES * 128, 64 * 3], f16,
                               addr_space="Shared")
            ar_in = dram.tile([128, 32 * 24], f32)
            ar_out = dram.tile([128, 32 * 24], f32, addr_space="Shared")

            # ================= phase A: conv1 + conv2 =================
            with tc.tile_pool(name="pA", bufs=1) as pA, \
                 tc.tile_pool(name="patch_p", bufs=4) as patch_p, \
                 tc.tile_pool(name="stfp", bufs=3) as stfp:
                NF = 6  # ring of padded frames; conv2 lags conv1 by 3
                X2L = NF * 3364
                x2p8 = pA.tile([128, X2L], f8)
                x2pr8 = pA.tile([128, X2L], f8)
                for t8, eng in ((x2p8, nc.scalar), (x2pr8, nc.gpsimd)):
                    for c0 in range(0, X2L, 4096):
                        n = min(4096, X2L - c0)
                        eng.dma_start(t8[:, c0:c0 + n], zsb[:, :n])
                x2p8v = x2p8[:].rearrange("p (d h w) -> p d h w",
                                          d=NF, h=58, w=58)
                x2pr8v = x2pr8[:].rearrange("p (d h w) -> p d h w",
                                            d=NF, h=58, w=58)
                w1_sb = pA.tile([108, 128], f16)
                nc.sync.dma_start(w1_sb[:], din["w1"][:])
                w2_sb = pA.tile([128, 10 * 2 * 128], f8)
                nc.sync.dma_start(w2_sb[:], din["w2dr"][:])
                w2v = w2_sb[:].rearrange("p (pr i m) -> p pr i m", pr=10,
                                         i=2)
                c2slots = _conv2_slot_pairs()

                # x3 frame staging (full padded 30x30 frame, borders zero)
                x3stg = [(stgp.tile([128, 900], f8, name=f"x3s8_{i}"),
                          stgp.tile([128, 900], f8, name=f"x3sr_{i}"))
                         for i in range(2)]
                for s8, sr in x3stg:
                    nc.vector.memset(s8[:], 0.0)
                    nc.gpsimd.memset(sr[:], 0.0)

                PXCLIP = SEGLEN * H0 * 56

                def conv1_half(clip, d, hh):
                    patch_sb = patch_p.tile([108, 7 * 448], f16, tag="patch")
                    nc.sync.dma_start(
                        patch_sb[:],
                        din["patches"][:, clip * PXCLIP + d * H0 * 56
                                       + hh * 3136:
                                       clip * PXCLIP + d * H0 * 56
                                       + hh * 3136 + 3136])
                    stF = stfp.tile([128, 7 * 448], f16, tag="stF")
                    for rg7 in range(7):
                        pt = ps.tile([128, 448], f32, tag="ps")
                        nc.tensor.matmul(
                            pt[:], w1_sb[:],
                            patch_sb[:, rg7 * 448:(rg7 + 1) * 448],
                            start=True, stop=True)
                        nc.scalar.activation(
                            stF[:, rg7 * 448:(rg7 + 1) * 448], pt[:],
                            AF.Relu, bias=bias_sb[:, 0:1], scale=float(BN))
                    # move upper 64 partitions down to base 0, then max in
                    # place (two-input engine ops need equal base partitions)
                    wp = stfp.tile([64, 7 * 448], f16, tag="wp")
                    nc.sync.dma_start(wp[:], stF[64:128, :])
                    nc.vector.tensor_tensor(wp[:], wp[:], stF[0:64, :],
                                            ALU.max)
                    wp4 = wp[:].rearrange("p (r i two w) -> p r i two w",
                                          r=7, i=4, two=2)
                    hp = stfp.tile([64, 28, 56], f16, tag="hp")
                    hp4 = hp[:].rearrange("p (r i) w -> p r i w", r=7)
                    nc.vector.tensor_tensor(hp4, wp4[:, :, :, 0],
                                            wp4[:, :, :, 1], ALU.max)
                    rows = slice(1 + hh * 28, 29 + hh * 28)
                    s1 = (clip * 17 + d + 1) % NF
                    d8 = x2p8v[0:64, s1, rows, 1:57]
                    if (d + hh) % 2 == 0:
                        nc.scalar.activation(d8, hp[:], AF.Copy)
                    else:
                        nc.vector.tensor_copy(d8, hp[:])
                    dr8 = x2pr8v[0:64, s1, rows, 1:57]
                    nc.vector.tensor_tensor(dr8, hp[:], d8, ALU.subtract)

                def conv2_rg2(clip, dd, rg2, stF2):
                    # two rg groups share one PSUM bank (row pitch 58 kept
                    # uniform: second group lands at offset 4*58=232)
                    pt = ps.tile([128, 462], f32, tag="ps")
                    for half in range(2):
                        rg = rg2 * 2 + half
                        def slot_off(s):
                            # ring-resolved flat offset of a conv2 slot
                            kd, kh, hf = s
                            return (((clip * 17 + dd + kd) % NF) * 3364
                                    + kh * 58 + hf + (rg * 4) * 58)
                        i = 0
                        for x2t in (x2p8, x2pr8):
                            for pr in range(10):
                                sa, sb = c2slots[pr]
                                oa = slot_off(sa)
                                delta = 0 if sb is None \
                                    else slot_off(sb) - oa
                                base = x2t[:, oa:oa + 1]
                                rhs = _ap_dims(base, [[delta, 2], [1, 230]])
                                nc.tensor.matmul(
                                    pt[:, half * 232:half * 232 + 230],
                                    w2v[:, pr], rhs,
                                    start=(i == 0), stop=(i == 19),
                                    perf_mode=DR)
                                i += 1
                    src = _ap_dims(pt[:, 0:1], [[58, 8], [1, 56]])
                    nc.scalar.activation(stF2[:, rg2 * 8:(rg2 + 1) * 8, :],
                                         src, AF.Relu,
                                         bias=bias_sb[:, 1:2],
                                         scale=float(BN))

                def conv2_frame_tail(clip, dd, stF2, dstage):
                    wpF = pool1.tile([128, 56, 28], f16, tag="wp2")
                    nc.vector.tensor_tensor(wpF[:], stF2[:, :, 0::2],
                                            stF2[:, :, 1::2], ALU.max)
                    hpF = dstp.tile([128, 28, 28], f16, tag="hp2")
                    nc.vector.tensor_tensor(hpF[:], wpF[:, 0::2, :],
                                            wpF[:, 1::2, :], ALU.max)
                    if dd % 2 == 0:
                        dstage["f"] = hpF
                        return
                    dpF = pool2.tile([128, 28, 28], f16, tag="dp")
                    nc.vector.tensor_tensor(dpF[:], dstage["f"][:], hpF[:],
                                            ALU.max)
                    s8, sr = x3stg[(dd // 2) % 2]
                    s8v = s8[:].rearrange("p (h w) -> p h w", h=30)
                    srv = sr[:].rearrange("p (h w) -> p h w", h=30)
                    t8 = s8v[:, 1:29, 1:29]
                    nc.scalar.activation(t8, dpF[:], AF.Copy)
                    nc.vector.tensor_tensor(srv[:, 1:29, 1:29], dpF[:], t8,
                                            ALU.subtract)

                LAG = 3
                dstage = {}
                for g in range(48 + LAG):
                    if g < 48:
                        clip, d = divmod(g, 16)
                        for hh in range(2):
                            conv1_half(clip, d, hh)
                        s1 = (clip * 17 + d + 1) % NF
                        # shifted upper-half copies (tap kw+2 packing)
                        nc.sync.dma_start(x2p8v[64:128, s1, 0:58, 0:56],
                                          x2p8v[0:64, s1, 0:58, 2:58])
                        nc.sync.dma_start(x2pr8v[64:128, s1, 0:58, 0:56],
                                          x2pr8v[0:64, s1, 0:58, 2:58])
                    if g >= LAG:
                        clip, dd = divmod(g - LAG, 16)
                        if dd == 13:
                            # re-zero the shared pad slot between clips
                            sp = ((clip + 1) * 17) % NF
                            nc.gpsimd.dma_start(
                                x2p8[:, sp * 3364:(sp + 1) * 3364],
                                zsb[:, :3364])
                            nc.gpsimd.dma_start(
                                x2pr8[:, sp * 3364:(sp + 1) * 3364],
                                zsb[:, :3364])
                        stF2 = stfp.tile([128, 56, 56], f16, tag="stF2")
                        for rg2 in range(7):
                            conv2_rg2(clip, dd, rg2, stF2)
                        conv2_frame_tail(clip, dd, stF2, dstage)
                        if dd % 2 == 1:
                            dout = dd // 2
                            s8, sr = x3stg[dout % 2]
                            v8, vr = vols["x3"][clip]
                            nc.sync.dma_start(
                                v8[:, (dout + 1) * 900:(dout + 2) * 900],
                                s8[:])
                            nc.sync.dma_start(
                                vr[:, (dout + 1) * 900:(dout + 2) * 900],
                                sr[:])

            zero_vols(["x3b", "x4", "x4b", "x5", "x5b"])

            # ================= phase B: conv3a .. conv5b =================
            def conv_layer(wname, invols, outvols):
                KB, MB, D, Hs, Ws, pool, bias_col, scale = CONVS[wname]
                PD, PH, PW = D + 2, Hs + 2, Ws + 2
                V = PD * PH * PW
                RG = 14 if Hs == 14 else 7
                n_rg = Hs // RG
                flat4 = Ws >= 14  # 4-dim no-waste windows (HW-validated
                # for these shapes; conv5's tiny windows stay 3-dim)
                N = RG * Ws if flat4 else (RG - 1) * PW + Ws
                taps = _tap_list(KB)
                offs = [((kb * PD + kd) * PH + kh) * PW + kw
                        for kb, kd, kh, kw in taps]
                ppairs = _dr_pairs(KB, PD, PH, PW)
                npr = len(ppairs)
                pairs = []
                for ta, tb in ppairs:
                    a = offs[ta]
                    b = offs[ta] if tb is None else offs[tb]
                    pairs.append((a, b - a))
                if pool == "222":
                    PHo, PWo = Hs // 2 + 2, Ws // 2 + 2
                    wps = [stgp.tile([128, Hs * (Ws // 2)], f16,
                                     name=f"{wname}_wps_{i}")
                           for i in range(2)]
                elif pool is None:
                    PHo, PWo = PH, PW
                if pool != "5":
                    stg = [(stgp.tile([128, PHo * PWo], f8,
                                      name=f"{wname}_s8_{i}"),
                            stgp.tile([128, PHo * PWo], f8,
                                      name=f"{wname}_sr_{i}"))
                           for i in range(3)]
                    for s8, sr in stg:
                        nc.vector.memset(s8[:], 0.0)
                        nc.gpsimd.memset(sr[:], 0.0)
                si = [0]
                WCOL = npr * 2 * 128
                for mb in range(MB):
                    wt = wpool.tile([128, WCOL], f8, tag="w")
                    nc.sync.dma_start(wt[:],
                                      din[wname][:, mb * WCOL:(mb + 1) * WCOL])
                    wtv = wt[:].rearrange("p (pr i m) -> p pr i m",
                                          pr=npr, i=2)
                    for clip in range(3):
                        x8t = xpool.tile([128, KB * V], f8, tag="x8")
                        nc.sync.dma_start(x8t[:], invols[clip][0][:])
                        r8t = xpool.tile([128, KB * V], f8, tag="r8")
                        nc.sync.dma_start(r8t[:], invols[clip][1][:])
                        dstage = {}
                        for d in range(D):
                            for rg in range(n_rg):
                                wbase = d * PH * PW + rg * RG * PW
                                pt = ps.tile([128, N], f32, tag="ps")
                                i = 0
                                nmm = 2 * npr
                                for xt in (x8t, r8t):
                                    for a, delta in pairs:
                                        base = xt[:, wbase + a:wbase + a + 1]
                                        rdims = [[delta, 2], [PW, RG],
                                                 [1, Ws]] if flat4 \
                                            else [[delta, 2], [1, N]]
                                        rhs = _ap_dims(base, rdims)
                                        nc.tensor.matmul(
                                            pt[:], wtv[:, i % npr], rhs,
                                            start=(i == 0),
                                            stop=(i == nmm - 1),
                                            perf_mode=DR)
                                        i += 1
                                y16 = stp.tile([128, RG, Ws], f16, tag="y")
                                src = _ap_dims(
                                    pt[:, 0:1],
                                    [[Ws if flat4 else PW, RG], [1, Ws]])
                                nc.scalar.activation(
                                    y16[:], src, AF.Relu,
                                    bias=bias_sb[:, bias_col + mb:
                                                 bias_col + mb + 1],
                                    scale=scale)
                                if pool is None:
                                    s8, sr = stg[si[0] % 3]
                                    s8v = s8[:].rearrange(
                                        "p (h w) -> p h w", h=PH)
                                    srv = sr[:].rearrange(
                                        "p (h w) -> p h w", h=PH)
                                    rr = slice(rg * RG + 1, rg * RG + 1 + RG)
                                    t8 = s8v[:, rr, 1:1 + Ws]
                                    ew().tensor_copy(t8, y16[:])
                                    ew().tensor_tensor(srv[:, rr, 1:1 + Ws],
                                                       y16[:], t8,
                                                       ALU.subtract)
                                elif pool == "222":
                                    wv = wps[d % 2][:].rearrange(
                                        "p (h w) -> p h w", h=Hs)
                                    ew().tensor_tensor(
                                        wv[:, rg * RG:rg * RG + RG, :],
                                        y16[:, :, 0::2], y16[:, :, 1::2],
                                        ALU.max)
                                else:  # conv5b: y16 is [128, 7, 7]
                                    if d % 2 == 0:
                                        dstage["f"] = y16
                                    else:
                                        dmx = pool1.tile([128, 7, 7], f16,
                                                         tag="dmx")
                                        ew().tensor_tensor(dmx[:],
                                                           dstage["f"][:],
                                                           y16[:], ALU.max)
                                        wp5 = pool2.tile([128, 7, 4], f16,
                                                         tag="wp5")
                                        nc.vector.tensor_copy(
                                            wp5[:, :, 0:1], dmx[:, :, 0:1])
                                        nc.vector.tensor_tensor(
                                            wp5[:, :, 1:4], dmx[:, :, 1:6:2],
                                            dmx[:, :, 2:7:2], ALU.max)
                                        hp5 = pool2.tile([128, 4, 4], f16,
                                                         tag="hp5")
                                        nc.vector.tensor_copy(
                                            hp5[:, 0:1, :], wp5[:, 0:1, :])
                                        nc.vector.tensor_tensor(
                                            hp5[:, 1:4, :], wp5[:, 1:6:2, :],
                                            wp5[:, 2:7:2, :], ALU.max)
                                        fv = featsd[:].rearrange(
                                            "p (m h w c) -> p m h w c",
                                            m=4, h=4, w=4)
                                        nc.sync.dma_start(fv[:, mb, :, :,
                                                             clip],
                                                          hp5[:])
                            # ---- end rg loop: frame-level epilogue ----
                            if pool is None:
                                s8, sr = stg[si[0] % 3]
                                si[0] += 1
                                v8, vr = outvols[clip]
                                fo = (mb * PD + d + 1) * PH * PW
                                nc.sync.dma_start(
                                    v8[:, fo:fo + PH * PW], s8[:])
                                nc.sync.dma_start(
                                    vr[:, fo:fo + PH * PW], sr[:])
                            elif pool == "222":
                                wv = wps[d % 2][:].rearrange(
                                    "p (h w) -> p h w", h=Hs)
                                hp = pool2.tile([128, Hs // 2, Ws // 2],
                                                f16, tag="hpL")
                                ew().tensor_tensor(hp[:], wv[:, 0::2, :],
                                                   wv[:, 1::2, :], ALU.max)
                                if d % 2 == 0:
                                    dstage["f"] = hp
                                else:
                                    dp = pool2.tile(
                                        [128, Hs // 2, Ws // 2], f16,
                                        tag="dpL")
                                    ew().tensor_tensor(dp[:],
                                                       dstage["f"][:],
                                                       hp[:], ALU.max)
                                    s8, sr = stg[si[0] % 3]
                                    si[0] += 1
                                    s8v = s8[:].rearrange(
                                        "p (h w) -> p h w", h=PHo)
                                    srv = sr[:].rearrange(
                                        "p (h w) -> p h w", h=PHo)
                                    hh = slice(1, 1 + Hs // 2)
                                    wwc = slice(1, 1 + Ws // 2)
                                    t8 = s8v[:, hh, wwc]
                                    ew().tensor_copy(t8, dp[:])
                                    ew().tensor_tensor(srv[:, hh, wwc],
                                                       dp[:], t8,
                                                       ALU.subtract)
                                    v8, vr = outvols[clip]
                                    dout = d // 2
                                    fo = (mb * (D // 2 + 2) + dout + 1) \
                                        * PHo * PWo
                                    nc.sync.dma_start(
                                        v8[:, fo:fo + PHo * PWo], s8[:])
                                    nc.sync.dma_start(
                                        vr[:, fo:fo + PHo * PWo], sr[:])

            fcp = ctx.enter_context(tc.tile_pool(name="fcp", bufs=1))
            with tc.tile_pool(name="wpool", bufs=3) as wpool, \
                 tc.tile_pool(name="xpool", bufs=2) as xpool:
                conv_layer("w3a", vols["x3"], vols["x3b"])
                conv_layer("w3b", vols["x3b"], vols["x4"])
                conv_layer("w4a", vols["x4"], vols["x4b"])
                conv_layer("w4b", vols["x4b"], vols["x5"])
            with tc.tile_pool(name="wpool", bufs=4) as wpool, \
                 tc.tile_pool(name="xpool", bufs=2) as xpool:
                f6w_sb = fcp.tile([128, 4 * 64 * 128], f16)
                for q in range(4):
                    nc.scalar.dma_start(
                        f6w_sb[:, q * 8192:(q + 1) * 8192],
                        din["fc6w"][:, q * 8192:(q + 1) * 8192])
                f7w_sb = fcp.tile([128, 32 * 4 * 128], f16)
                for q in range(4):
                    nc.scalar.dma_start(
                        f7w_sb[:, q * 4096:(q + 1) * 4096],
                        din["fc7w"][:, q * 4096:(q + 1) * 4096])
                conv_layer("w5a", vols["x5"], vols["x5b"])
                conv_layer("w5b", vols["x5b"], None)

            # ================= phase C: FC + gram + sinkhorn =================
            # One AllGather (conv features) + fc6 sharded over 512 outputs;
            # fc7 is linear in the fc6 blocks, so each core computes fc7
            # partials from its local fc6 slice and one AllReduce replaces
            # the other two gathers.
            nc.gpsimd.collective_compute(
                "AllGather", ALU.bypass,
                replica_groups=[list(range(N_CORES))],
                ins=[featsd.opt()], outs=[ag1out.opt()])

            if True:
                rhs6 = fcp.tile([128, 64, 24], f16)
                agv = ag1out[:].rearrange("(r p) k -> r p k", r=8)
                r6v = rhs6[:].rearrange("p kb (r cl) -> p kb r cl", r=8)
                for r in range(8):
                    nc.sync.dma_start(r6v[:, :, r], agv[r])
                a6 = fcp.tile([128, 4, 24], f16)
                for mb in range(4):
                    pt = ps.tile([128, 24], f32, tag="ps")
                    for kb in range(64):
                        nc.tensor.matmul(
                            pt[:], f6w_sb[:, (mb * 64 + kb) * 128:
                                          (mb * 64 + kb + 1) * 128],
                            rhs6[:, kb], start=(kb == 0), stop=(kb == 63))
                    nc.scalar.activation(a6[:, mb], pt[:], AF.Relu,
                                         bias=bias_sb[:, 22 + mb:23 + mb],
                                         scale=float(BN))
                arst = fcp.tile([128, 32, 24], f32)
                for omb in range(32):
                    pt = ps.tile([128, 24], f32, tag="ps")
                    for kbl in range(4):
                        nc.tensor.matmul(
                            pt[:], f7w_sb[:, (omb * 4 + kbl) * 128:
                                          (omb * 4 + kbl + 1) * 128],
                            a6[:, kbl], start=(kbl == 0), stop=(kbl == 3))
                    if omb % 2:
                        nc.vector.tensor_copy(arst[:, omb], pt[:])
                    else:
                        nc.scalar.activation(arst[:, omb], pt[:], AF.Copy)
                nc.sync.dma_start(ar_in[:], arst[:])
                nc.gpsimd.collective_compute(
                    "AllReduce", ALU.add,
                    replica_groups=[list(range(N_CORES))],
                    ins=[ar_in.opt()], outs=[ar_out.opt()])

                arsb = arst
                nc.sync.dma_start(arsb[:], ar_out[:])
                fr = fcp.tile([128, 32, 24], f16)
                for omb in range(32):
                    nc.scalar.activation(fr[:, omb], arsb[:, omb], AF.Relu,
                                         bias=bias_sb[:, 26 + omb:27 + omb],
                                         scale=float(BN))
                gps = ps.tile([24, 24], f32, tag="ps")
                for kb in range(32):
                    nc.tensor.matmul(gps[:], fr[:, kb], fr[:, kb],
                                     start=(kb == 0), stop=(kb == 31))

                g_sb = sk.tile([24, 24], f32)
                nc.vector.tensor_copy(g_sb[:], gps[:])
                gdram = dram.tile([24, 24], f32)
                nc.sync.dma_start(gdram[:], g_sb[:])
                gflat = gdram[:].rearrange("a b -> (a b)")
                dg = sk.tile([1, 24], f32)
                nc.sync.dma_start(dg[:], gflat[None, ::25])
                sq = sk.tile([1, 24], f32)
                nc.scalar.activation(sq[:], dg[:], AF.Sqrt)
                nc.vector.tensor_scalar_add(sq[:], sq[:], 1e-8)
                inv = sk.tile([1, 24], f32)
                nc.vector.reciprocal(inv[:], sq[:])
                invd = dram.tile([1, 24], f32)
                nc.sync.dma_start(invd[:], inv[:])
                inv_col = sk.tile([24, 1], f32)
                nc.sync.dma_start(inv_col[:],
                                  invd[:].rearrange("a b -> (a b)")[:, None])
                t1 = sk.tile([24, 24], f32)
                nc.vector.tensor_scalar_mul(t1[:], g_sb[:], inv_col[:])
                eye_sb = sk.tile([24, 24], f32)
                nc.sync.dma_start(eye_sb[:], din["eye24"][:])
                tps = ps.tile([24, 24], f32, tag="ps")
                nc.tensor.transpose(tps[:], t1[:], eye_sb[:])
                t2 = sk.tile([24, 24], f32)
                nc.vector.tensor_copy(t2[:], tps[:])
                cos_sb = sk.tile([24, 24], f32)
                nc.vector.tensor_scalar_mul(cos_sb[:], t2[:], inv_col[:])
                cosd = dram.tile([24, 24], f32)
                nc.sync.dma_start(cosd[:], cos_sb[:])

                cos_ij = sk.tile([9, 4, 4], f32)
                for qv in range(3):
                    for sv in range(3):
                        p = qv * 3 + sv
                        src = cosd[:][None, 12 + qv * 4:12 + qv * 4 + 4,
                                      sv * 4:sv * 4 + 4]
                        nc.sync.dma_start(cos_ij[p:p + 1], src)

                bmat_sb = sk.tile([9, 4, 4], f32)
                nc.sync.dma_start(
                    bmat_sb[:],
                    din["bmat"][:].rearrange("p (i j) -> p i j", i=4))
                arg = sk.tile([9, 4, 4], f32)
                nc.vector.tensor_scalar_mul(arg[:], cos_ij[:], float(REG))
                nc.vector.tensor_tensor(arg[:], arg[:], bmat_sb[:], ALU.add)
                kt = sk.tile([9, 4, 4], f32)
                nc.scalar.activation(kt[:], arg[:], AF.Exp)
                ktT = sk.tile([9, 4, 4], f32)
                nc.vector.tensor_copy(ktT[:],
                                      kt[:].rearrange("p i j -> p j i"))
                sem = sk.tile([9, 4, 4], f32)
                nc.vector.tensor_scalar(sem[:], cos_ij[:], -1.0, 1.0,
                                        ALU.mult, ALU.add)
                msem = sk.tile([9, 4, 4], f32)
                nc.vector.tensor_tensor(msem[:], kt[:], sem[:], ALU.mult)

                u = sk.tile([9, 4], f32)
                nc.vector.memset(u[:], 0.25)
                prod = sk.tile([9, 4, 4], f32)
                s = sk.tile([9, 4], f32)
                v = sk.tile([9, 4], f32)
                EPS4 = 4e-9
                for it in range(SINK_ITERS + 1):
                    nc.vector.tensor_tensor(
                        prod[:], ktT[:],
                        u[:, None, :].broadcast_to([9, 4, 4]), ALU.mult)
                    nc.vector.reduce_sum(s[:, :, None], prod[:],
                                         axis=mybir.AxisListType.X)
                    nc.vector.tensor_scalar_add(s[:], s[:], EPS4)
                    nc.vector.reciprocal(v[:], s[:])
                    if it == SINK_ITERS:
                        break
                    nc.vector.tensor_tensor(
                        prod[:], kt[:],
                        v[:, None, :].broadcast_to([9, 4, 4]), ALU.mult)
                    nc.vector.reduce_sum(s[:, :, None], prod[:],
                                         axis=mybir.AxisListType.X)
                    nc.vector.tensor_scalar_add(s[:], s[:], EPS4)
                    nc.vector.reciprocal(u[:], s[:])

                ta = sk.tile([9, 4, 4], f32)
                nc.vector.tensor_tensor(
                    ta[:], msem[:],
                    u[:, :, None].broadcast_to([9, 4, 4]), ALU.mult)
                nc.vector.tensor_tensor(
                    ta[:], ta[:],
                    v[:, None, :].broadcast_to([9, 4, 4]), ALU.mult)
                t9s = sk.tile([9, 1], f32)
                nc.vector.reduce_sum(t9s[:, :, None], ta[:],
                                     axis=mybir.AxisListType.XY)
                o9 = sk.tile([9, 1], f32)
                nc.scalar.mul(o9[:], t9s[:], -0.25)
                nc.sync.dma_start(out_d[:], o9[:])

    nc.compile()
    return nc


def kernel(**inputs):
    from concourse.bass_utils import run_bass_kernel_spmd
    if "nc" not in _BUILD_CACHE:
        _BUILD_CACHE["nc"] = _build()
    nc = _BUILD_CACHE["nc"]
    in_maps = _prep_inputs(inputs)
    res = run_bass_kernel_spmd(nc, in_maps, core_ids=list(range(N_CORES)))
    return res.results[0]["out"].reshape(3, 3).astype(np.float32)
